# revision 7
# baseline (speedup 1.0000x reference)
"""Trainium2 Bass kernel for nn_Conv2d_20590073217670.

Conv2d: input [32,64,64,64] (NCHW), weight [576,128] (unfold layout:
row = ci*9 + a*3 + b for tap (a,b)), bias [1,128,1,1], stride 1, pad 1.
Output [32,128,64,64].

Strategy: data-parallel over batch - 4 images per NeuronCore, 8 cores.
All matmuls run in bf16 (4x the fp32r PE rate); the rel-err budget
(2e-2) dwarfs bf16 rounding (~3e-3 measured).  Host converts inputs to
bf16 and upcasts the bf16 output back to fp32.

Implicit GEMM over the 9 taps with K=128 tap-pairing.  Tiles are DENSE
[128, 64, 64] (8 KiB/partition) so every DMA is a single contiguous
chunk per partition; the +-1-column / +-1-row tap shifts are realized
as 1- and 63-element shifts of the flattened per-partition image
vector, sourced straight from HBM (partition-shifted halves) or DVE
(same-partition shift), with wrap-garbage columns zeroed by tiny
memsets (= the conv zero-pad border semantics):
  Tc: parts 0:64 = img[r,c], parts 64:128 = img[r,c+1] (col 63 -> 0)
  Td: parts 0:64 = img[r,c-1] (col 0 -> 0),
      parts 64:128 = img[r+1,c-1] (col 0 -> 0)
Per 8-row output block, 5 full-width matmuls accumulate one PSUM bank:
  (a,1)+(a,2) pairs on Tc for a=0,1,2; (0,0)+(1,0) pair on Td; and a
  K=64 single (2,0) on Td's lower half.  The matmul sweep runs
  weight-slot-major over 4-block half-images so consecutive matmuls
  share the stationary operand (amortizes LDWEIGHTS).  Vertical
  borders restrict output rows (PSUM has_written keeps partial sums
  exact; each bank's first matmul covers it fully).  ScalarE evicts
  4-bank PSUM tiles with a fused bias add to bf16.
"""
import sys

for _p in ("/opt/trn_rl_repo", "/root/.axon_site/_ro/trn_rl_repo"):
    if _p not in sys.path:
        sys.path.append(_p)

import numpy as np
import ml_dtypes
from contextlib import ExitStack

import concourse.bacc as bacc
import concourse.tile as tile
from concourse import mybir
from concourse.bass_utils import run_bass_kernel_spmd

f32 = mybir.dt.float32
bf16 = mybir.dt.bfloat16

N_CORES = 8
NB = 4  # images per core


def build_nc():
    nc = bacc.Bacc()
    x = nc.declare_dram_parameter("x", [NB, 64, 64, 64], bf16, isOutput=False)
    w = nc.declare_dram_parameter("w", [576, 128], bf16, isOutput=False)
    bias = nc.declare_dram_parameter("b", [128, 1], f32, isOutput=False)
    out = nc.declare_dram_parameter("out", [NB, 128, 64, 64], bf16, isOutput=True)

    with tile.TileContext(nc) as tc, ExitStack() as ctx:
        const = ctx.enter_context(tc.tile_pool(name="const", bufs=1))
        tc_pool = ctx.enter_context(tc.tile_pool(name="tc", bufs=NB))
        td_pool = ctx.enter_context(tc.tile_pool(name="td", bufs=NB))
        ob_pool = ctx.enter_context(tc.tile_pool(name="ob", bufs=4))
        ps_pool = ctx.enter_context(tc.tile_pool(name="ps", bufs=2, space="PSUM"))

        # ---- weights: [128, 5, 128]; slot s pairs tap u (parts 0:64) with
        # tap l (parts 64:128), taps indexed t = 3a + b:
        #   slot 0: (0,1)+(0,2)   slot 1: (1,1)+(1,2)   slot 2: (2,1)+(2,2)
        #   slot 3: (0,0)+(1,0)   slot 4: (1,0)+(2,0)
        w3 = w[:].rearrange("(c t) m -> c t m", t=9)
        WT = const.tile([128, 5, 128], bf16)
        bt = const.tile([128, 1], f32)
        for s, (u, l) in enumerate(((1, 2), (4, 5), (7, 8), (0, 3), (3, 6))):
            nc.sync.dma_start(out=WT[0:64, s, :], in_=w3[:, u, :])
            nc.sync.dma_start(out=WT[64:128, s, :], in_=w3[:, l, :])
        nc.sync.dma_start(out=bt[:], in_=bias[:])

        act_id = mybir.ActivationFunctionType.Identity

        for n in range(NB):
            Tc = tc_pool.tile([128, 64, 64], bf16)
            Td = td_pool.tile([128, 64, 64], bf16)
            Tcf = Tc[:].rearrange("p r c -> p (r c)")
            Tdf = Td[:].rearrange("p r c -> p (r c)")
            xf = x[n].rearrange("c r w -> c (r w)")
            # contiguous flat loads; partition-shifted halves come straight
            # from HBM, the same-partition shift (Td upper) goes via DVE.
            # Each stream is split at row 34 so the first half-image's
            # matmuls (which read rows <= 33) start before the tail lands.
            M = 34 * 64
            nc.sync.dma_start(out=Tcf[0:64, 0:M], in_=xf[:, 0:M])
            nc.sync.dma_start(out=Tcf[64:128, 0:M], in_=xf[:, 1:M + 1])
            nc.sync.dma_start(out=Tdf[64:128, 0:M], in_=xf[:, 63:M + 63])
            nc.vector.tensor_copy(Tdf[0:64, 1:M], Tcf[0:64, 0:M - 1])
            nc.vector.memset(Tc[64:128, 0:34, 63:64], 0.0)
            nc.vector.memset(Td[0:64, 0:34, 0:1], 0.0)
            nc.vector.memset(Td[64:128, 0:34, 0:1], 0.0)
            nc.sync.dma_start(out=Tcf[0:64, M:4096], in_=xf[:, M:4096])
            nc.sync.dma_start(out=Tcf[64:128, M:4095], in_=xf[:, M + 1:4096])
            nc.sync.dma_start(out=Tdf[64:128, M:4033], in_=xf[:, M + 63:4096])
            nc.vector.tensor_copy(Tdf[0:64, M:4096], Tcf[0:64, M - 1:4095])
            nc.vector.memset(Tc[64:128, 34:64, 63:64], 0.0)
            nc.vector.memset(Td[0:64, 34:64, 0:1], 0.0)
            nc.vector.memset(Td[64:128, 34:64, 0:1], 0.0)

            for half in range(2):
                blks = range(half * 4, half * 4 + 4)
                r0 = half * 32
                P = ps_pool.tile([128, 32, 64], f32)  # 4 PSUM banks
                osb = ob_pool.tile([128, 32, 64], bf16)

                def pr(blk, lo=0, hi=8):
                    q0 = (blk % 4) * 8
                    return P[:, q0 + lo:q0 + hi, :]

                # slot-major sweep; slot 1 first: full coverage on every bank
                for blk in blks:
                    y0 = blk * 8
                    nc.tensor.matmul(pr(blk), WT[:, 1, :], Tc[:, y0:y0 + 8, :],
                                     start=True, stop=False)
                for blk in blks:
                    y0 = blk * 8
                    if blk == 0:
                        nc.tensor.matmul(pr(blk, 1, 8), WT[:, 0, :],
                                         Tc[:, 0:7, :], start=False, stop=False)
                    else:
                        nc.tensor.matmul(pr(blk), WT[:, 0, :],
                                         Tc[:, y0 - 1:y0 + 7, :],
                                         start=False, stop=False)
                for blk in blks:
                    y0 = blk * 8
                    if blk == 7:
                        nc.tensor.matmul(pr(blk, 0, 7), WT[:, 2, :],
                                         Tc[:, 57:64, :], start=False, stop=False)
                    else:
                        nc.tensor.matmul(pr(blk), WT[:, 2, :],
                                         Tc[:, y0 + 1:y0 + 9, :],
                                         start=False, stop=False)
                # slot 3: dp01 pairs (blk >= 1), then blk 0's (0,0) single
                for blk in blks:
                    y0 = blk * 8
                    if blk != 0:
                        nc.tensor.matmul(pr(blk), WT[:, 3, :],
                                         Td[:, y0 - 1:y0 + 7, :],
                                         start=False, stop=False)
                if half == 0:
                    nc.tensor.matmul(pr(0, 1, 8), WT[0:64, 3, :],
                                     Td[0:64, 0:7, :], start=False, stop=False)
                    # slot 4: blk 0's dp12 pair (its last), then (2,0) singles
                    nc.tensor.matmul(pr(0), WT[:, 4, :], Td[:, 0:8, :],
                                     start=False, stop=True)
                for blk in blks:
                    y0 = blk * 8
                    if blk == 0:
                        continue
                    if blk == 7:
                        nc.tensor.matmul(pr(blk, 0, 7), WT[64:128, 4, :],
                                         Td[64:128, 56:63, :],
                                         start=False, stop=True)
                    else:
                        nc.tensor.matmul(pr(blk), WT[64:128, 4, :],
                                         Td[64:128, y0:y0 + 8, :],
                                         start=False, stop=True)

                nc.scalar.activation(osb[:], P[:], act_id, bias=bt[:])
                nc.sync.dma_start(out=out[n][:, r0:r0 + 32, :], in_=osb[:])

    nc.finalize()
    return nc


_NC = None


def _get_nc():
    global _NC
    if _NC is None:
        _NC = build_nc()
    return _NC


def kernel(**inputs) -> np.ndarray:
    x = np.ascontiguousarray(
        np.asarray(inputs["input"], dtype=np.float32)).astype(ml_dtypes.bfloat16)
    w = np.ascontiguousarray(
        np.asarray(inputs["weight"], dtype=np.float32)).astype(ml_dtypes.bfloat16)
    b = np.ascontiguousarray(
        np.asarray(inputs["bias"], dtype=np.float32).reshape(128, 1))
    nc = _get_nc()
    in_maps = [
        {"x": x[c * NB:(c + 1) * NB], "w": w, "b": b} for c in range(N_CORES)
    ]
    res = run_bass_kernel_spmd(nc, in_maps, list(range(N_CORES)))
    full = np.concatenate([r["out"] for r in res.results], axis=0)
    return full.astype(np.float32)
